# revision 1
# baseline (speedup 1.0000x reference)
"""Trainium2 Bass kernel for nn_DetectModel (RGAT x3 + TopK pool + MLP).

ap_gather costs ~28ns/index regardless of payload width, so per-edge work
is restructured to minimize gather *indices*:
  - One fused table index c3 = (a_dst*114 + t)*10 + a_src (11400 entries).
  - Phase A: one d=4 fp16 gather/edge serves layers 0+1 ((U0,P0,U1,P1) where
    U_l = Mt_l[s]*P_l, P_l = exp-lrelu attention factor).
  - Phase B: one d=2 fp16 gather/edge for layer 2.
  - Rank/alive masks computed in wrapped [128, Lp/16] space (no gather),
    expanded to [128, Lp] via a DRAM round-trip DMA transpose.
  - Tables built at full 128-partition width from replicated [*,1140] factors.
  - fp16 edge streams + fp16 identity PE matmuls into f32 PSUM.
  - Inputs shipped wrapped/int16+f32 (~0.7MB/core vs ~5.6MB in v1).
"""
import numpy as np

import concourse.bass as bass
import concourse.bacc as bacc
import concourse.mybir as mybir
import concourse.tile as tile
from concourse.bass_utils import run_bass_kernel_spmd
from concourse.masks import make_identity

F32 = mybir.dt.float32
F16 = mybir.dt.float16
I16 = mybir.dt.int16
AF = mybir.ActivationFunctionType
OP = mybir.AluOpType
AX = mybir.AxisListType

N0, N1, N2 = 50000, 40000, 32000
E = 600000
D = 16
R = 114
NA = 10
NCORES = 8
G = 8
NB = NCORES * G

NE3 = 11408            # c3 table slots; dead index = 11400
DEAD3 = 11400
QUOTAS = (N0, N1, N2)
TQ3 = ((0, 380), (380, 760), (760, 1140))


# ---------------------------------------------------------------- host prep

def host_prep(node_ids, edge_index, edge_type):
    a = np.asarray(node_ids).astype(np.int64)
    src = np.asarray(edge_index)[0].astype(np.int64)
    dst = np.asarray(edge_index)[1].astype(np.int64)
    t = np.asarray(edge_type).astype(np.int64)
    n0 = a.shape[0]

    cnt = np.bincount(a, minlength=NA).astype(np.int64)
    order_a = np.argsort(a, kind="stable")
    rank = np.empty(n0, np.int64)
    starts = np.zeros(NA + 1, np.int64)
    np.cumsum(cnt, out=starts[1:])
    rank[order_a] = np.arange(n0) - starts[a[order_a]]

    deg = np.bincount(dst, minlength=n0)
    e_order = np.argsort(dst, kind="stable")
    vstart = np.zeros(n0 + 1, np.int64)
    np.cumsum(deg, out=vstart[1:])

    vorder = np.argsort(-deg, kind="stable")
    nrows = (n0 + NB - 1) // NB
    pad_n = nrows * NB
    vpad = np.full(pad_n, -1, np.int64)
    vpad[:n0] = vorder
    grid = vpad.reshape(nrows, NB)
    grid[1::2] = grid[1::2, ::-1]
    buckets = [grid[:, b][grid[:, b] >= 0] for b in range(NB)]

    nv_eq = max(len(b) for b in buckets)
    nv_eq = ((nv_eq + 15) // 16) * 16

    maxdeg = int(deg.max()) if n0 else 0
    n_k = []
    for k in range(maxdeg):
        w = max(int((deg[b] > k).sum()) for b in buckets)
        if w == 0:
            break
        n_k.append(w)
    n_k[0] = nv_eq
    offs = np.zeros(len(n_k) + 1, np.int64)
    np.cumsum(n_k, out=offs[1:])
    L = int(offs[-1])
    Lp = ((L + 255) // 256) * 256

    c3_e = ((a[dst] * R + t) * NA + a[src]).astype(np.int64)
    z_src_e = (a[src] * 8192 + rank[src]).astype(np.float32)

    def wrap(x, width):
        return (x.reshape(G, width // 16, 16)
                 .transpose(0, 2, 1)
                 .reshape(G * 16, width // 16))

    in_maps = []
    for n in range(NCORES):
        c3Idx = np.full((G, Lp), DEAD3, np.int64)
        zS = np.full((G, Lp), 8191.0, np.float32)
        zD = np.full((G, nv_eq), 8191.0, np.float32)
        for g in range(G):
            bl = buckets[n * G + g]
            nb = len(bl)
            zD[g, :nb] = (a[bl] * 8192 + rank[bl]).astype(np.float32)
            dg = deg[bl]
            for k in range(len(n_k)):
                rsel = np.nonzero(dg > k)[0]
                if len(rsel) == 0:
                    continue
                eids = e_order[vstart[bl[rsel]] + k]
                cols = offs[k] + rsel
                c3Idx[g, cols] = c3_e[eids]
                zS[g, cols] = z_src_e[eids]

        m = {
            "c3IdxW": np.ascontiguousarray(wrap(c3Idx, Lp).astype(np.int16)),
            "zSrcB": np.ascontiguousarray(zS.reshape(128, Lp // 16)),
            "zDstB": np.ascontiguousarray(zD.reshape(128, nv_eq // 16)),
        }
        in_maps.append(m)

    meta = {"n_k": n_k, "offs": offs, "L": L, "Lp": Lp, "nv_eq": nv_eq,
            "cnt": cnt}
    return in_maps, meta


def pack_model_inputs(inp, cnt):
    f = lambda x: np.ascontiguousarray(np.asarray(x, np.float32))
    emb = f(inp["emb"])
    sh = {
        "embT": np.ascontiguousarray(emb.T),
        "pw0": f(inp["pw0"]).reshape(16, 1),
        "pw1": f(inp["pw1"]).reshape(16, 1),
        "cntRep": np.ascontiguousarray(np.tile(cnt.astype(np.float32), (NA, 1))),
        "cntCol": cnt.astype(np.float32).reshape(NA, 1),
        "l1w": f(inp["l1w"]), "l1b": f(inp["l1b"]).reshape(1, 16),
        "l2w": f(inp["l2w"]), "l2b": f(inp["l2b"]).reshape(1, 4),
        "l3w": f(inp["l3w"]), "l3b": f(inp["l3b"]).reshape(1, 1),
    }
    for l in range(3):
        W = f(inp[f"W{l}"])
        sh[f"Wst{l}"] = np.ascontiguousarray(
            W.transpose(1, 0, 2).reshape(16, R * 16))
        sh[f"qv{l}"] = f(inp[f"q{l}"]).reshape(16, 1)
        sh[f"kv{l}"] = f(inp[f"k{l}"]).reshape(16, 1)
        sh[f"bv{l}"] = f(inp[f"b{l}"]).reshape(16, 1)
    return sh


def seg_matmul_list(meta, n_chunks):
    n_k, offs, Lp = meta["n_k"], meta["offs"], meta["Lp"]
    Wc = Lp // n_chunks
    out = []
    for k, nk in enumerate(n_k):
        lo, hi = int(offs[k]), int(offs[k] + nk)
        p = lo
        pieces = []
        while p < hi:
            q = min(hi, (p // Wc + 1) * Wc)
            r0, r1 = p - lo, q - lo
            if r0 < 512 < r1:
                pieces += [(p, lo + 512), (lo + 512, q)]
            else:
                pieces.append((p, q))
            p = q
        for (p0, p1) in pieces:
            out.append({
                "chunk": p0 // Wc,
                "rhs_lo": p0 - (p0 // Wc) * Wc,
                "rhs_hi": p1 - (p0 // Wc) * Wc,
                "ps_lo": p0 - lo,
                "start": (k == 0),
            })
    return out


# ------------------------------------------------------------- bass builder

def build_program(meta, n_chunks=16, repeats=1, skip=(), num_devices=NCORES,
                  tail=True):
    n_k, Lp, nv_eq = meta["n_k"], meta["Lp"], meta["nv_eq"]
    Wc = Lp // n_chunks
    assert Wc % 16 == 0
    W16 = Lp // 16
    NV16 = nv_eq // 16
    segs = seg_matmul_list(meta, n_chunks)
    by_chunk = {}
    for e in segs:
        by_chunk.setdefault(e["chunk"], []).append(e)

    nc = bacc.Bacc("TRN2", target_bir_lowering=False, debug=False,
                   num_devices=num_devices)

    din = {}
    din["c3IdxW"] = nc.dram_tensor("c3IdxW", [128, W16], I16, kind="ExternalInput")
    din["zSrcB"] = nc.dram_tensor("zSrcB", [128, W16], F32, kind="ExternalInput")
    din["zDstB"] = nc.dram_tensor("zDstB", [128, NV16], F32, kind="ExternalInput")
    for name, shape in [("embT", [16, NA]), ("pw0", [16, 1]), ("pw1", [16, 1]),
                        ("cntRep", [NA, NA]), ("cntCol", [NA, 1]),
                        ("l1w", [96, 16]), ("l1b", [1, 16]),
                        ("l2w", [16, 4]), ("l2b", [1, 4]),
                        ("l3w", [4, 1]), ("l3b", [1, 1])]:
        din[name] = nc.dram_tensor(name, shape, F32, kind="ExternalInput")
    for l in range(3):
        din[f"Wst{l}"] = nc.dram_tensor(f"Wst{l}", [16, R * 16], F32, kind="ExternalInput")
        for nm in ("qv", "kv", "bv"):
            din[f"{nm}{l}"] = nc.dram_tensor(f"{nm}{l}", [16, 1], F32, kind="ExternalInput")
    dout = nc.dram_tensor("out", [1], F32, kind="ExternalOutput")

    with tile.TileContext(nc) as tc:
        with (
            tc.tile_pool(name="stat", bufs=1) as stat,
            tc.tile_pool(name="tabp", bufs=1) as tabp,
            tc.tile_pool(name="wrk", bufs=2) as wrk,
            tc.tile_pool(name="tiny", bufs=1) as tiny,
            tc.tile_pool(name="dram", bufs=1, space="DRAM") as dram,
        ):
            # ---------- static loads ----------
            c3Idx = stat.tile([128, W16], I16)
            zSrcB = stat.tile([128, W16], F32)
            zDstB = stat.tile([128, NV16], F32)
            embT = stat.tile([16, NA], F32)
            cntRep = stat.tile([NA, NA], F32)
            cntCol = stat.tile([NA, 1], F32)
            l1w = stat.tile([96, 16], F32)
            l1b = stat.tile([1, 16], F32)
            l2w = stat.tile([16, 4], F32)
            l2b = stat.tile([1, 4], F32)
            l3w = stat.tile([4, 1], F32)
            l3b = stat.tile([1, 1], F32)
            pw = [stat.tile([16, 1], F32, tag=f"pw{i}", name=f"pw{i}") for i in range(2)]
            qv = [stat.tile([16, 1], F32, tag=f"qv{i}", name=f"qvt{i}") for i in range(3)]
            kv = [stat.tile([16, 1], F32, tag=f"kv{i}", name=f"kvt{i}") for i in range(3)]
            for tl, name in ([(c3Idx, "c3IdxW"), (zSrcB, "zSrcB"),
                              (zDstB, "zDstB"),
                              (embT, "embT"), (cntRep, "cntRep"), (cntCol, "cntCol"),
                              (l1w, "l1w"), (l1b, "l1b"), (l2w, "l2w"), (l2b, "l2b"),
                              (l3w, "l3w"), (l3b, "l3b"),
                              (pw[0], "pw0"), (pw[1], "pw1")]
                             + [(qv[l], f"qv{l}") for l in range(3)]
                             + [(kv[l], f"kv{l}") for l in range(3)]):
                nc.sync.dma_start(out=tl[:], in_=din[name].ap())
            bRep = [stat.tile([128, 1], F32, tag=f"bR{i}", name=f"bR{i}") for i in range(3)]
            for l in range(3):
                nc.sync.dma_start(out=bRep[l][:],
                                  in_=bass.AP(din[f"bv{l}"], 0, [[0, 8], [1, 16]]))

            ident = stat.tile([128, 128], F32)
            make_identity(nc, ident[:])
            identF = stat.tile([128, 128], F16)
            make_identity(nc, identF[:])
            ones1 = stat.tile([1, 128], F32)
            nc.vector.memset(ones1[:], 1.0)

            def colbc(col_ap, n, m):
                return bass.AP(col_ap.tensor, col_ap.offset,
                               [list(col_ap.ap[0]), [0, m]])

            # ---------- prolog: s0, s1, keep counts, kTab ----------
            with tc.tile_pool(name="psP", bufs=1, space="PSUM") as psP:
                def ps_small(shape, tg="tps"):
                    return psP.tile(shape, F32, space="PSUM", tag=tg, name="pstiny")

                def rnorm_of(pwt):
                    nrm = ps_small([1, 1])
                    nc.tensor.matmul(out=nrm[:], lhsT=pwt[:], rhs=pwt[:],
                                     start=True, stop=True)
                    sq = tiny.tile([1, 1], F32, tag="sq")
                    nc.scalar.activation(out=sq[:], in_=nrm[:], func=AF.Sqrt)
                    rn = tiny.tile([1, 1], F32, tag="rn")
                    nc.vector.reciprocal(out=rn[:], in_=sq[:])
                    rrep = ps_small([NA, 1])
                    nc.tensor.matmul(out=rrep[:], lhsT=ones1[:, :NA], rhs=rn[:],
                                     start=True, stop=True)
                    rs = tiny.tile([NA, 1], F32, tag="rs10")
                    nc.vector.tensor_copy(out=rs[:], in_=rrep[:])
                    return rs

                rn0 = rnorm_of(pw[0])
                s0 = stat.tile([NA, 1], F32)
                dot0 = ps_small([NA, 1])
                nc.tensor.matmul(out=dot0[:], lhsT=embT[:], rhs=pw[0][:],
                                 start=True, stop=True)
                nc.scalar.activation(out=s0[:], in_=dot0[:], func=AF.Tanh, scale=rn0[:])
                rn1 = rnorm_of(pw[1])
                dot1 = ps_small([NA, 1])
                nc.tensor.matmul(out=dot1[:], lhsT=embT[:], rhs=pw[1][:],
                                 start=True, stop=True)
                d1s = tiny.tile([NA, 1], F32, tag="d1s")
                nc.vector.tensor_tensor(out=d1s[:], in0=dot1[:], in1=s0[:], op=OP.mult)
                s1 = stat.tile([NA, 1], F32)
                nc.scalar.activation(out=s1[:], in_=d1s[:], func=AF.Tanh, scale=rn1[:])

                cc = [stat.tile([NA, 1], F32, tag=f"cc{i}", name=f"cct{i}") for i in range(3)]
                nc.vector.memset(cc[0][:], 1.0)
                nc.vector.tensor_copy(out=cc[1][:], in_=s0[:])
                nc.vector.tensor_tensor(out=cc[2][:], in0=s0[:], in1=s1[:], op=OP.mult)

                def keep_counts(score_col, quota, prev_col, kk):
                    srow_ps = ps_small([1, NA])
                    nc.tensor.transpose(out=srow_ps[:], in_=score_col[:],
                                        identity=ident[:NA, :NA])
                    srow = tiny.tile([1, NA], F32, tag="srow")
                    nc.vector.tensor_copy(out=srow[:], in_=srow_ps[:])
                    srep_ps = ps_small([NA, NA])
                    nc.tensor.matmul(out=srep_ps[:], lhsT=ones1[:, :NA], rhs=srow[:],
                                     start=True, stop=True)
                    gt = tiny.tile([NA, NA], F32, tag="gt")
                    nc.vector.tensor_tensor(out=gt[:], in0=srep_ps[:],
                                            in1=colbc(score_col[:], NA, NA), op=OP.is_gt)
                    nc.vector.tensor_tensor(out=gt[:], in0=gt[:], in1=cntRep[:], op=OP.mult)
                    cum = tiny.tile([NA, 1], F32, tag="cum")
                    nc.vector.tensor_reduce(cum[:], gt[:], AX.X, OP.add)
                    nc.vector.tensor_scalar(out=kk[:], in0=cum[:], scalar1=-1.0,
                                            scalar2=float(quota), op0=OP.mult, op1=OP.add)
                    nc.vector.tensor_scalar(out=kk[:], in0=kk[:], scalar1=0.0,
                                            scalar2=None, op0=OP.max)
                    nc.vector.tensor_tensor(out=kk[:], in0=kk[:], in1=prev_col[:], op=OP.min)

                kcol = [cntCol,
                        stat.tile([NA, 1], F32, tag="k1", name="k1"),
                        stat.tile([NA, 1], F32, tag="k2", name="k2")]
                keep_counts(s0, N1, kcol[0], kcol[1])
                keep_counts(s1, N2, kcol[1], kcol[2])

                kTab = []
                for l in range(3):
                    kr = stat.tile([1, 16], F32, tag=f"kr{l}", name=f"kr{l}")
                    nc.vector.memset(kr[:], 0.0)
                    kr_ps = ps_small([1, NA])
                    nc.tensor.transpose(out=kr_ps[:], in_=kcol[l][:],
                                        identity=ident[:NA, :NA])
                    nc.vector.tensor_copy(out=kr[:, :NA], in_=kr_ps[:])
                    kt_ps = ps_small([128, 16], tg="ktps")
                    nc.tensor.matmul(out=kt_ps[:], lhsT=ones1[:], rhs=kr[:],
                                     start=True, stop=True)
                    kt = stat.tile([128, 16], F32, tag=f"kt{l}", name=f"ktt{l}")
                    nc.vector.tensor_copy(out=kt[:], in_=kt_ps[:])
                    kTab.append(kt)

            # ---------- alive via interval counting on z = a*8192 + rank ----------
            # alive(z; k) = sum_a [z < a*8192 + k[a]] - sum_a [z < a*8192]
            baseTab = stat.tile([128, 16], F32)
            for a in range(NA):
                nc.vector.memset(baseTab[:, a:a + 1], float(a * 8192))
            KTab = []
            for l in range(3):
                ktl = stat.tile([128, 16], F32, tag=f"KT{l}", name=f"KT{l}")
                nc.vector.tensor_tensor(out=ktl[:, :NA], in0=kTab[l][:, :NA],
                                        in1=baseTab[:, :NA], op=OP.add)
                KTab.append(ktl)

            def cbc(t, col, m):
                return bass.AP(t[:].tensor, t[:].offset + col,
                               [list(t[:].ap[0]), [0, m]])

            def count_lt(zt, ktab, ncols, dtype, tag):
                acc = tiny.tile([128, ncols], dtype, tag=f"clt_{tag}",
                                name=f"clt{tag}")
                stp = tiny.tile([128, ncols], dtype, tag=f"stp_{tag}",
                                name=f"stp{tag}")
                for a in range(NA):
                    dst = acc if a == 0 else stp
                    nc.vector.tensor_tensor(out=dst[:], in0=zt[:],
                                            in1=cbc(ktab, a, ncols), op=OP.is_lt)
                    if a > 0:
                        nc.vector.tensor_tensor(out=acc[:], in0=acc[:],
                                                in1=stp[:], op=OP.add)
                return acc

            c3mD = stat.tile([128, W16], F32)
            nc.vector.tensor_copy(out=c3mD[:], in_=c3Idx[:])
            nc.vector.tensor_scalar(out=c3mD[:], in0=c3mD[:],
                                    scalar1=float(-DEAD3), scalar2=None,
                                    op0=OP.add)
            baseCntS = stat.tile([128, W16], F16)
            tmpS = count_lt(zSrcB, baseTab, W16, F16, "s")
            nc.vector.tensor_copy(out=baseCntS[:], in_=tmpS[:])
            baseCntD = stat.tile([128, NV16], F32)
            tmpD = count_lt(zDstB, baseTab, NV16, F32, "d")
            nc.vector.tensor_copy(out=baseCntD[:], in_=tmpD[:])

            # dram scratch for wrapped->expanded transposes
            scrW = dram.tile([128, W16], F16)
            scrD = dram.tile([128, NV16], F32)

            def expand(wr_tile, ncols, out_tile, dscr):
                # SBUF block [128, ncols] -> DRAM -> SBUF expanded [128, 16*ncols]
                # (partition 16g+p holds cols [p*ncols, (p+1)*ncols) of bucket g)
                nc.sync.dma_start(out=dscr[:, :ncols], in_=wr_tile[:])
                for g in range(G):
                    src = bass.AP(dscr[:].tensor,
                                  dscr[:].offset + g * 16 * dscr.shape[1],
                                  [[0, 16], [dscr.shape[1], 16], [1, ncols]])
                    o = out_tile[16 * g:16 * (g + 1), :]
                    dst = bass.AP(o.tensor, o.offset,
                                  [list(o.ap[0]), [ncols, 16], [1, ncols]])
                    nc.sync.dma_start(out=dst, in_=src)

            def build_alive(zt, l, baseCnt, ncols, out_dtype, tag):
                aliveW = count_lt(zt, KTab[l], ncols, out_dtype, tag)
                nc.vector.tensor_tensor(out=aliveW[:], in0=aliveW[:],
                                        in1=baseCnt[:], op=OP.subtract)
                return aliveW

            # static dst mask for layer 0 (rank < cnt[a]: real-vertex indicator)
            aliveD0 = stat.tile([128, nv_eq], F32)
            alD0w = build_alive(zDstB, 0, baseCntD, NV16, F32, "d")
            expand(alD0w, NV16, aliveD0, scrD)

            partials = stat.tile([128, 8], F32)
            nc.vector.memset(partials[:], 0.0)

            # big fused table + per-layer factor tiles
            tabT = tabp.tile([128, NE3 * 4], F16)
            MtS = tabp.tile([128, 1140], F16, tag="mts")
            EQ = tabp.tile([128, 1140], F16, tag="eq")
            E2Q = tabp.tile([128, 1140], F16, tag="e2q")
            EK = tabp.tile([128, 1140], F16, tag="ek")
            E2K = tabp.tile([128, 1140], F16, tag="e2k")
            wst = tabp.tile([16, R * 16], F32, tag="wst")
            aliveB = tabp.tile([128, Lp], F16, tag="aliveB")
            aliveD = [tabp.tile([128, nv_eq], F32, tag=f"alivD{l}",
                                name=f"alivD{l}") for l in (1, 2)]

            def replicate16(t, width):
                # [0:16, :width] -> all 128 partitions (doubling DMAs)
                for p in (16, 32, 64):
                    nc.sync.dma_start(out=t[p:2 * p, :width], in_=t[0:p, :width])

            def build_layer_factors(l, psT):
                # emblT = embT * cc[l] (broadcast over features)
                crow_ps = psT.tile([1, NA], F32, space="PSUM", tag="crow")
                nc.tensor.transpose(out=crow_ps[:], in_=cc[l][:],
                                    identity=ident[:NA, :NA])
                crow = tiny.tile([1, NA], F32, tag="crow")
                nc.vector.tensor_copy(out=crow[:], in_=crow_ps[:])
                crep_ps = psT.tile([16, NA], F32, space="PSUM", tag="crep")
                nc.tensor.matmul(out=crep_ps[:], lhsT=ones1[:, :16],
                                 rhs=crow[:], start=True, stop=True)
                emblT = tiny.tile([16, NA], F32, tag="emblT")
                nc.vector.tensor_tensor(out=emblT[:], in0=embT[:],
                                        in1=crep_ps[:], op=OP.mult)

                nc.sync.dma_start(out=wst[:], in_=din[f"Wst{l}"].ap())
                qvR = tiny.tile([16, 16], F16, tag="qvR")
                kvR = tiny.tile([16, 16], F16, tag="kvR")
                nc.vector.tensor_copy(out=qvR[:], in_=colbc(qv[l][:], 16, 16))
                nc.vector.tensor_copy(out=kvR[:], in_=colbc(kv[l][:], 16, 16))

                # Mt [16, 1140] (s = t*10 + a_src)
                for blk in range(3):
                    mt_ps = psT.tile([16, 512], F32, space="PSUM", tag="mt")
                    t0 = blk * 38
                    t1 = min(R, t0 + 38)
                    for ti in range(t0, t1):
                        col = (ti - t0) * NA
                        nc.tensor.matmul(
                            out=mt_ps[:, col:col + NA],
                            lhsT=wst[:, ti * 16:(ti + 1) * 16],
                            rhs=emblT[:],
                            start=True, stop=True, skip_group_check=True)
                    cw = (t1 - t0) * NA
                    nc.vector.tensor_copy(out=MtS[:16, t0 * NA:t0 * NA + cw],
                                          in_=mt_ps[:, :cw])

                # Tq/Tk + exps (s-space)
                for (c0, c1) in TQ3:
                    tkp = psT.tile([16, 512], F32, space="PSUM", tag="tqk")
                    nc.tensor.matmul(out=tkp[:, :c1 - c0], lhsT=kvR[:],
                                     rhs=MtS[:16, c0:c1], start=True, stop=True,
                                     skip_group_check=True)
                    nc.scalar.activation(out=EK[:16, c0:c1], in_=tkp[:, :c1 - c0],
                                         func=AF.Exp)
                    nc.scalar.activation(out=E2K[:16, c0:c1], in_=tkp[:, :c1 - c0],
                                         func=AF.Exp, scale=0.2)
                    tqp = psT.tile([16, 512], F32, space="PSUM", tag="tqk2")
                    nc.tensor.matmul(out=tqp[:, :c1 - c0], lhsT=qvR[:],
                                     rhs=MtS[:16, c0:c1], start=True, stop=True,
                                     skip_group_check=True)
                    nc.scalar.activation(out=EQ[:16, c0:c1], in_=tqp[:, :c1 - c0],
                                         func=AF.Exp)
                    nc.scalar.activation(out=E2Q[:16, c0:c1], in_=tqp[:, :c1 - c0],
                                         func=AF.Exp, scale=0.2)
                for tt in (MtS, EQ, E2Q, EK, E2K):
                    replicate16(tt, 1140)

            def write_planes(d_total, slot_u, slot_p):
                # P = max(EQ[c2]*EK[s], E2Q[c2]*E2K[s]); U = Mt[s]*P, written
                # interleaved into tabT at stride d_total (full 128-partition).
                Ablk = tiny.tile([128, 1140], F16, tag="Ablk")
                Bblk = tiny.tile([128, 1140], F16, tag="Bblk")
                for a0 in range(NA):
                    def qview(t):
                        return bass.AP(t[:].tensor, t[:].offset + a0,
                                       [list(t[:].ap[0]), [NA, R], [0, NA]])
                    def sview(t):
                        return bass.AP(t[:].tensor, t[:].offset,
                                       [list(t[:].ap[0]), [NA, R], [1, NA]])
                    nc.vector.tensor_tensor(out=Ablk[:], in0=qview(EQ),
                                            in1=sview(EK), op=OP.mult)
                    nc.vector.tensor_tensor(out=Bblk[:], in0=qview(E2Q),
                                            in1=sview(E2K), op=OP.mult)
                    nc.vector.tensor_tensor(out=Ablk[:], in0=Ablk[:], in1=Bblk[:],
                                            op=OP.max)
                    base = a0 * 1140 * d_total
                    pout = bass.AP(tabT[:].tensor, tabT[:].offset + base + slot_p,
                                   [list(tabT[:].ap[0]), [d_total, 1140]])
                    nc.vector.tensor_copy(out=pout, in_=Ablk[:])
                    uout = bass.AP(tabT[:].tensor, tabT[:].offset + base + slot_u,
                                   [list(tabT[:].ap[0]), [d_total, 1140]])
                    nc.vector.tensor_tensor(out=uout, in0=sview(MtS), in1=Ablk[:],
                                            op=OP.mult)

            def run_phase(psX, d_total, layers, tail_specs, idx_tile=None):
                # layers: list of (slot_u, slot_p, aliveB or None) streams
                w1 = min(512, nv_eq)
                w2 = nv_eq - w1
                acc = {}
                for li, _ in enumerate(layers):
                    for nm in ("U", "S"):
                        tiles = [psX.tile([128, w1], F32, space="PSUM",
                                          tag=f"ps{nm}{li}0", name=f"ps{nm}{li}0")]
                        if w2 > 0:
                            tiles.append(psX.tile([128, w2], F32, space="PSUM",
                                                  tag=f"ps{nm}{li}1", name=f"ps{nm}{li}1"))
                        acc[(li, nm)] = tiles

                last_e = {}
                first_e = {}
                for ci in range(n_chunks):
                    for e in by_chunk.get(ci, []):
                        tx = 0 if e["ps_lo"] < 512 else 1
                        last_e[tx] = id(e)
                        if tx not in first_e:
                            first_e[tx] = id(e)

                for ci in range(n_chunks):
                    i0 = ci * (Wc // 16)
                    sO = wrk.tile([128, Wc * d_total], F16, tag=f"sO{d_total}",
                                  name="sO")
                    if "gather" not in skip:
                        nc.gpsimd.ap_gather(
                            out_ap=sO[:].rearrange("p (w d) -> p w d", d=d_total),
                            in_ap=tabT[:, :NE3 * d_total].rearrange(
                                "p (s d) -> p s d", d=d_total),
                            idxs_ap=(idx_tile if idx_tile is not None
                                     else c3Idx)[:, i0:i0 + Wc // 16],
                            channels=128, num_elems=NE3, d=d_total, num_idxs=Wc)
                    else:
                        nc.vector.memset(sO[:, :128], 1.0)
                    sv = sO[:].rearrange("p (w d) -> p w d", d=d_total)
                    streams = []
                    for li, (slot_u, slot_p, alB) in enumerate(layers):
                        if alB is None:
                            streams.append((sv[:, :, slot_u], sv[:, :, slot_p]))
                        else:
                            bt = wrk.tile([128, Wc], F16, tag=f"bt{li}",
                                          name=f"bt{li}")
                            m1 = wrk.tile([128, Wc], F16, tag=f"m1{li}",
                                          name=f"m1{li}")
                            als = alB[:, ci * Wc:(ci + 1) * Wc]
                            nc.vector.tensor_tensor(out=bt[:], in0=sv[:, :, slot_u],
                                                    in1=als, op=OP.mult)
                            nc.vector.tensor_tensor(out=m1[:], in0=sv[:, :, slot_p],
                                                    in1=als, op=OP.mult)
                            streams.append((bt[:], m1[:]))

                    for e in by_chunk.get(ci, []):
                        pl = e["ps_lo"]
                        tix = 0 if pl < 512 else 1
                        pb = pl - tix * 512
                        wdt = e["rhs_hi"] - e["rhs_lo"]
                        is_last = last_e.get(tix) == id(e)
                        is_first = first_e.get(tix) == id(e)
                        if "pe" in skip and not (is_first or is_last):
                            continue
                        for li, (bt, m1) in enumerate(streams):
                            nc.tensor.matmul(
                                out=acc[(li, "U")][tix][:, pb:pb + wdt],
                                lhsT=identF[:],
                                rhs=bt[:, e["rhs_lo"]:e["rhs_hi"]],
                                start=is_first, stop=is_last,
                                skip_group_check=True)
                            nc.tensor.matmul(
                                out=acc[(li, "S")][tix][:, pb:pb + wdt],
                                lhsT=identF[:],
                                rhs=m1[:, e["rhs_lo"]:e["rhs_hi"]],
                                start=is_first, stop=is_last,
                                skip_group_check=True)

                # tails
                for li, (l, alD) in enumerate(tail_specs):
                    psU = acc[(li, "U")]
                    psS = acc[(li, "S")]
                    Svec = tiny.tile([128, nv_eq], F32, tag="svec")
                    for tix in range(len(psU)):
                        c0 = tix * 512
                        cw = psS[tix].shape[1]
                        nc.vector.tensor_scalar(out=Svec[:, c0:c0 + cw],
                                                in0=psS[tix][:], scalar1=1e-16,
                                                scalar2=None, op0=OP.add)
                    nc.vector.reciprocal(out=Svec[:], in_=Svec[:])
                    h = tiny.tile([128, nv_eq], F32, tag="h")
                    for tix in range(len(psU)):
                        c0 = tix * 512
                        cw = psU[tix].shape[1]
                        nc.vector.tensor_tensor(out=h[:, c0:c0 + cw],
                                                in0=psU[tix][:],
                                                in1=Svec[:, c0:c0 + cw], op=OP.mult)
                    nc.scalar.activation(out=h[:], in_=h[:], func=AF.Relu,
                                         bias=bRep[l][:])
                    nc.vector.tensor_tensor(out=h[:], in0=h[:], in1=alD[:],
                                            op=OP.mult)
                    nc.vector.tensor_reduce(partials[:, l:l + 1], h[:], AX.X, OP.add)
                    nc.vector.tensor_reduce(partials[:, 3 + l:4 + l], h[:], AX.X,
                                            OP.max)

            # ---------------- repeats ----------------
            for _rep in range(repeats):
                # per-layer alive masks (dst for l=1,2; src for l=1,2)
                alW1 = build_alive(zSrcB, 1, baseCntS, W16, F16, "s")
                expand(alW1, W16, aliveB, scrW)
                alD1w = build_alive(zDstB, 1, baseCntD, NV16, F32, "d")
                expand(alD1w, NV16, aliveD[0], scrD)
                alD2w = build_alive(zDstB, 2, baseCntD, NV16, F32, "d")
                expand(alD2w, NV16, aliveD[1], scrD)

                # phase A: layers 0+1
                with tc.tile_pool(name=f"psT{_rep}", bufs=1, space="PSUM") as psT:
                    build_layer_factors(0, psT)
                    write_planes(4, 0, 1)
                    build_layer_factors(1, psT)
                    write_planes(4, 2, 3)
                    nc.vector.memset(tabT[:, DEAD3 * 4:], 0.0)
                with tc.tile_pool(name=f"psA{_rep}", bufs=1, space="PSUM") as psA:
                    run_phase(psA, 4,
                              [(0, 1, None), (2, 3, aliveB)],
                              [(0, aliveD0), (1, aliveD[0])])

                # phase B: layer 2 (d=2 table in the same tile)
                with tc.tile_pool(name=f"psU{_rep}", bufs=1, space="PSUM") as psT2:
                    build_layer_factors(2, psT2)
                alW2 = build_alive(zSrcB, 2, baseCntS, W16, F32, "s2")
                c3X = tiny.tile([128, W16], I16, tag="c3X")
                c3Xf = tiny.tile([128, W16], F32, tag="c3Xf")
                nc.vector.tensor_tensor(out=c3Xf[:], in0=c3mD[:], in1=alW2[:],
                                        op=OP.mult)
                nc.vector.tensor_scalar(out=c3X[:], in0=c3Xf[:],
                                        scalar1=float(DEAD3), scalar2=None,
                                        op0=OP.add)
                write_planes(2, 0, 1)
                nc.vector.memset(tabT[:, DEAD3 * 2:NE3 * 2], 0.0)
                with tc.tile_pool(name=f"psB{_rep}", bufs=1, space="PSUM") as psB:
                    run_phase(psB, 2,
                              [(0, 1, None)],
                              [(2, aliveD[1])], idx_tile=c3X)

                # ---------------- combine + MLP ----------------
                if not tail:
                    continue
                with tc.tile_pool(name=f"psM{_rep}", bufs=1, space="PSUM") as psM:
                    def ps_small2(shape, tg="tps"):
                        return psM.tile(shape, F32, space="PSUM", tag=tg, name="pstiny")

                    cc_in = dram.tile([128, 8], F32)
                    cc_out = dram.tile([NCORES * 128, 8], F32)
                    nc.sync.dma_start(out=cc_in[:], in_=partials[:])
                    nc.gpsimd.collective_compute(
                        "AllGather", OP.bypass,
                        replica_groups=[list(range(NCORES))],
                        ins=[cc_in[:].opt()], outs=[cc_out[:].opt()])
                    allp = tiny.tile([128, NCORES * 8], F32, tag="allp")
                    nc.sync.dma_start(
                        out=allp[:],
                        in_=bass.AP(cc_out[:].tensor, cc_out[:].offset,
                                    [[8, 128], [1024, NCORES], [1, 8]]))
                    comb = tiny.tile([128, 8], F32, tag="comb")
                    nc.vector.memset(comb[:], 0.0)
                    ab = allp[:]
                    nc.vector.tensor_reduce(
                        comb[:, 0:3],
                        bass.AP(ab.tensor, ab.offset,
                                [list(ab.ap[0]), [1, 3], [8, NCORES]]),
                        AX.X, OP.add)
                    nc.vector.tensor_reduce(
                        comb[:, 3:6],
                        bass.AP(ab.tensor, ab.offset + 3,
                                [list(ab.ap[0]), [1, 3], [8, NCORES]]),
                        AX.X, OP.max)
                    shf = tiny.tile([128, 8], F32, tag="shf")
                    for sh in (64, 32, 16):
                        nc.sync.dma_start(out=shf[:sh, :], in_=comb[sh:2 * sh, :])
                        nc.vector.tensor_tensor(out=comb[:sh, 0:3], in0=comb[:sh, 0:3],
                                                in1=shf[:sh, 0:3], op=OP.add)
                        nc.vector.tensor_tensor(out=comb[:sh, 3:6], in0=comb[:sh, 3:6],
                                                in1=shf[:sh, 3:6], op=OP.max)
                    for l in range(3):
                        nc.vector.tensor_scalar(out=comb[:16, l:l + 1],
                                                in0=comb[:16, l:l + 1],
                                                scalar1=1.0 / QUOTAS[l], scalar2=None,
                                                op0=OP.mult)
                    gcol = tiny.tile([96, 1], F32, tag="gcol")
                    for l in range(3):
                        nc.sync.dma_start(out=gcol[32 * l:32 * l + 16, :],
                                          in_=comb[:16, l:l + 1])
                        nc.sync.dma_start(out=gcol[32 * l + 16:32 * l + 32, :],
                                          in_=comb[:16, 3 + l:4 + l])
                    z1_ps = ps_small2([1, 16])
                    nc.tensor.matmul(out=z1_ps[:], lhsT=gcol[:], rhs=l1w[:],
                                     start=True, stop=True)
                    h1 = tiny.tile([1, 16], F32, tag="h1")
                    nc.vector.tensor_tensor(out=h1[:], in0=z1_ps[:], in1=l1b[:], op=OP.add)
                    nc.scalar.activation(out=h1[:], in_=h1[:], func=AF.Relu)
                    h1c_ps = ps_small2([16, 1], tg="h1c")
                    nc.tensor.transpose(out=h1c_ps[:], in_=h1[:], identity=ident[:1, :1])
                    h1c = tiny.tile([16, 1], F32, tag="h1c")
                    nc.vector.tensor_copy(out=h1c[:], in_=h1c_ps[:])
                    z2_ps = ps_small2([1, 4], tg="z2")
                    nc.tensor.matmul(out=z2_ps[:], lhsT=h1c[:], rhs=l2w[:],
                                     start=True, stop=True)
                    h2 = tiny.tile([1, 4], F32, tag="h2")
                    nc.vector.tensor_tensor(out=h2[:], in0=z2_ps[:], in1=l2b[:], op=OP.add)
                    nc.scalar.activation(out=h2[:], in_=h2[:], func=AF.Relu)
                    h2c_ps = ps_small2([4, 1], tg="h2c")
                    nc.tensor.transpose(out=h2c_ps[:], in_=h2[:], identity=ident[:1, :1])
                    h2c = tiny.tile([4, 1], F32, tag="h2c")
                    nc.vector.tensor_copy(out=h2c[:], in_=h2c_ps[:])
                    z3_ps = ps_small2([1, 1], tg="z3")
                    nc.tensor.matmul(out=z3_ps[:], lhsT=h2c[:], rhs=l3w[:],
                                     start=True, stop=True)
                    z3 = tiny.tile([1, 1], F32, tag="z3")
                    nc.vector.tensor_tensor(out=z3[:], in0=z3_ps[:], in1=l3b[:], op=OP.add)
                    sig = tiny.tile([1, 1], F32, tag="sig")
                    nc.scalar.activation(out=sig[:], in_=z3[:], func=AF.Sigmoid)
                    nc.sync.dma_start(out=dout.ap(), in_=sig[:])
            if not tail:
                nc.sync.dma_start(out=dout.ap(), in_=partials[:1, :1])

    nc.finalize()
    return nc


# ------------------------------------------------------------------ driver

_CACHE = {}


def kernel(**inputs):
    in_maps_nc, meta = host_prep(inputs["node_ids"], inputs["edge_index"],
                                 inputs["edge_type"])
    shared = pack_model_inputs(inputs, meta["cnt"])
    in_maps = [{**m, **shared} for m in in_maps_nc]

    key = (meta["Lp"], meta["nv_eq"], tuple(meta["n_k"]))
    if key not in _CACHE:
        _CACHE[key] = build_program(meta)
    nc = _CACHE[key]

    res = run_bass_kernel_spmd(nc, in_maps, core_ids=list(range(NCORES)))
    return np.asarray(res.results[0]["out"], np.float32)



# revision 5
# speedup vs baseline: 1.4601x; 1.4601x over previous
"""Trainium2 Bass kernel for nn_DetectModel (RGAT x3 + TopK pool + MLP).

v2: SWDGE dma_gather (transpose mode) replaces gpsimd ap_gather.

Since x = emb[node_ids] has only 10 distinct rows and TopK scales are
per-attribute scalars, every per-edge quantity depends only on
c3 = (a_dst*114 + t)*10 + a_src (11400 combos).  A DRAM table holds, per
c3 and per alive1-state, a 128-plane row:
  planes 0:16 U0 = Mt0[s]*P0    16:32 U1*alive1    32:48 U2
  plane  48 P2   49 P0   50 P1   rest zero
(alive1-dead edges index a second bank of rows with U1/P1/U2/P2 zeroed;
alive2 masking is a DVE multiply on planes 32:49 with a host mask row.)

One dma_gather(transpose=True, elem=256B) per 512-vertex block fetches
each edge's row as a column: out[plane, edge].  Segment softmax sums
become PSUM accumulation over degree-prefix runs (vertices in each block
sorted by degree, so run k = first n_k columns — zero padding).  All
three layers' numerators and denominators accumulate in a single
[128, 512] PSUM tile per block; tails do recip/broadcast/relu/mask and
mean/max partials; one AllGather combines cores; tiny MLP finishes.

TopK keep-sets are replicated on host in numpy (scores take 10 distinct
values; selection is exact argsort replication of jax.lax.top_k).
"""
import numpy as np

import concourse.bass as bass
import concourse.bacc as bacc
import concourse.mybir as mybir
import concourse.tile as tile
from concourse.bass_utils import run_bass_kernel_spmd
from concourse.masks import make_identity

F32 = mybir.dt.float32
F16 = mybir.dt.float16
I16 = mybir.dt.int16
AF = mybir.ActivationFunctionType
OP = mybir.AluOpType
AX = mybir.AxisListType

N0, N1, N2 = 50000, 40000, 32000
E = 600000
D = 16
R = 114
NA = 10
NEG = 0.2
NCORES = 8

NC3 = R * NA * NA          # 11400
BANK = 11408               # state-0 rows start here
ZR = 2 * BANK              # all-zero row
NROWSP = ZR + 32           # padded table rows
BLK = 512                  # vertices per block / psum tile width


# ---------------------------------------------------------------- host prep

def host_prep(node_ids, edge_index, edge_type, emb, W0, q0, k0, W1, q1, k1,
              W2, q2, k2, pw0, pw1, **_unused):
    a = np.asarray(node_ids).astype(np.int64)
    src = np.asarray(edge_index)[0].astype(np.int64)
    dst = np.asarray(edge_index)[1].astype(np.int64)
    t = np.asarray(edge_type).astype(np.int64)
    emb = np.asarray(emb, np.float32)
    pw0 = np.asarray(pw0, np.float32).reshape(-1)
    pw1 = np.asarray(pw1, np.float32).reshape(-1)

    # ---- replicate reference TopK pooling on host (10 distinct scores) ----
    s0a = np.tanh((emb @ pw0) / np.linalg.norm(pw0))          # [10]
    score0 = s0a[a]
    perm1 = np.argsort(-score0, kind="stable")[:N1]
    keep1 = np.zeros(N0, bool)
    keep1[perm1] = True
    s1a = np.tanh(s0a * (emb @ pw1) / np.linalg.norm(pw1))    # [10]
    score1 = s1a[a[perm1]]
    perm2 = np.argsort(-score1, kind="stable")[:N2]
    keep2 = np.zeros(N0, bool)
    keep2[perm1[perm2]] = True

    m1 = keep1[src] & keep1[dst]
    m2 = m1 & keep2[src] & keep2[dst]

    # ---- per-edge fused index ----
    c3_e = (a[dst] * R + t) * NA + a[src]
    idx_e = np.where(m1, c3_e, c3_e + BANK).astype(np.int32)

    # ---- table [NROWSP, 128] f16 ----
    cc = np.stack([np.ones(NA, np.float32), s0a, s0a * s1a])  # [3, 10]
    tab = np.zeros((NROWSP, 128), np.float32)
    Ws = [np.asarray(W0, np.float32), np.asarray(W1, np.float32),
          np.asarray(W2, np.float32)]
    qs = [np.asarray(q0, np.float32).reshape(-1),
          np.asarray(q1, np.float32).reshape(-1),
          np.asarray(q2, np.float32).reshape(-1)]
    ks = [np.asarray(k0, np.float32).reshape(-1),
          np.asarray(k1, np.float32).reshape(-1),
          np.asarray(k2, np.float32).reshape(-1)]
    pslot = (49, 50, 48)   # P0, P1, P2 plane slots
    for l in range(3):
        xl = emb * cc[l][:, None]                              # [10, 16]
        XW = np.einsum("ad,tdk->tak", xl, Ws[l])               # [114, 10, 16]
        Mt = XW.reshape(R * NA, D)                             # s = t*10+a
        Tq = XW @ qs[l]                                        # [114, 10] (a_dst)
        Tk = (XW @ ks[l]).reshape(R * NA)                      # [1140] (s)
        z3 = (Tq.T[:, :, None] + Tk.reshape(1, R, NA)).reshape(NA, R * NA)
        P = np.exp(np.where(z3 > 0, z3, NEG * z3))             # [10, 1140]
        U = Mt[None, :, :] * P[:, :, None]                     # [10, 1140, 16]
        tab[:NC3, 16 * l:16 * l + 16] = U.reshape(NC3, D)
        tab[:NC3, pslot[l]] = P.reshape(NC3)
    # state-0 bank: only layer-0 planes survive
    tab[BANK:BANK + NC3, 0:16] = tab[:NC3, 0:16]
    tab[BANK:BANK + NC3, 49] = tab[:NC3, 49]
    tab16 = tab.astype(np.float16)

    # ---- vertex -> core assignment (degree snake) ----
    deg = np.bincount(dst, minlength=N0)
    e_order = np.argsort(dst, kind="stable")
    vstart = np.zeros(N0 + 1, np.int64)
    np.cumsum(deg, out=vstart[1:])

    vorder = np.argsort(-deg, kind="stable")
    nrows = (N0 + NCORES - 1) // NCORES
    vpad = np.full(nrows * NCORES, -1, np.int64)
    vpad[:N0] = vorder
    grid = vpad.reshape(nrows, NCORES)
    grid[1::2] = grid[1::2, ::-1]
    vlists = [grid[:, c][grid[:, c] >= 0] for c in range(NCORES)]
    nv = max(len(v) for v in vlists)
    NB = (nv + BLK - 1) // BLK
    NVP = NB * BLK

    # unified run widths per block: n_k[b][k] = max over cores
    degs = [deg[v] for v in vlists]
    blocks = []           # list of (widths list incl. first=BLK, pad)
    tot = 0
    for b in range(NB):
        maxd = 0
        for c in range(NCORES):
            db = degs[c][b * BLK:(b + 1) * BLK]
            if len(db):
                maxd = max(maxd, int(db.max()))
        widths = []
        for k in range(maxd):
            w = 0
            for c in range(NCORES):
                db = degs[c][b * BLK:(b + 1) * BLK]
                w = max(w, int((db > k).sum()))
            if w == 0:
                break
            widths.append(w)
        if not widths:
            widths = [1]
        widths[0] = BLK
        wsum = sum(widths)
        pad = (-wsum) % 128
        blocks.append((widths, pad))
        tot += wsum + pad
    TOT = tot

    # ---- per-core edge streams ----
    in_maps = []
    for c in range(NCORES):
        v = vlists[c]
        dgc = degs[c]
        idxs = np.full(TOT, ZR, np.int32)
        m2s = np.zeros(TOT, np.float16)
        pos = 0
        for b in range(NB):
            vb = v[b * BLK:(b + 1) * BLK]
            db = dgc[b * BLK:(b + 1) * BLK]
            widths, pad = blocks[b]
            for k, w in enumerate(widths):
                nreal = int((db > k).sum()) if len(db) else 0
                if nreal:
                    eids = e_order[vstart[vb[:nreal]] + k]
                    idxs[pos:pos + nreal] = idx_e[eids]
                    m2s[pos:pos + nreal] = m2[eids]
                pos += w
            pos += pad
        assert pos == TOT
        # wrap indices: flat j -> partition j%16, col j//16; replicate x8
        iw = np.zeros((16, TOT // 16), np.int16)
        iw[:, :] = idxs.reshape(TOT // 16, 16).T
        idxW = np.tile(iw, (8, 1))

        aliveD = np.zeros((48, NVP), np.float16)
        nreal_v = len(v)
        aliveD[0:16, :nreal_v] = 1.0
        aliveD[16:32, :nreal_v] = keep1[v][None, :]
        aliveD[32:48, :nreal_v] = keep2[v][None, :]

        in_maps.append({
            "idxW": np.ascontiguousarray(idxW),
            "m2row": np.ascontiguousarray(m2s.reshape(1, TOT)),
            "aliveD": np.ascontiguousarray(aliveD),
            "tabD": tab16,
        })

    meta = {"blocks": blocks, "TOT": TOT, "NVP": NVP, "NB": NB}
    return in_maps, meta


def pack_model_inputs(inp, meta=None):
    f = lambda x: np.ascontiguousarray(np.asarray(x, np.float32))
    b48 = np.zeros((48, 1), np.float32)
    for l in range(3):
        b48[16 * l:16 * l + 16, 0] = np.asarray(inp[f"b{l}"], np.float32)
    selS = np.zeros((51, 48), np.float32)
    selS[49, 0:16] = 1.0
    selS[50, 16:32] = 1.0
    selS[48, 32:48] = 1.0
    nscale = np.zeros((48, 1), np.float32)
    for l, nl in enumerate((N1, N2, N0)):
        pass
    nscale[0:16, 0] = 1.0 / N0
    nscale[16:32, 0] = 1.0 / N1
    nscale[32:48, 0] = 1.0 / N2
    return {
        "bias48": b48, "selSD": selS, "nscale": nscale,
        "l1w": f(inp["l1w"]), "l1b": f(inp["l1b"]).reshape(1, 16),
        "l2w": f(inp["l2w"]), "l2b": f(inp["l2b"]).reshape(1, 4),
        "l3w": f(inp["l3w"]), "l3b": f(inp["l3b"]).reshape(1, 1),
    }


# ------------------------------------------------------------- bass builder

def build_program(meta, repeats=1, num_devices=NCORES):
    blocks, TOT, NVP, NB = (meta["blocks"], meta["TOT"], meta["NVP"],
                            meta["NB"])
    WMAX = max(sum(w for w in ws) + pad for ws, pad in blocks)

    nc = bacc.Bacc("TRN2", target_bir_lowering=False, debug=False,
                   num_devices=num_devices)

    din = {}
    din["tabD"] = nc.dram_tensor("tabD", [NROWSP, 128], F16, kind="ExternalInput")
    din["idxW"] = nc.dram_tensor("idxW", [128, TOT // 16], I16, kind="ExternalInput")
    din["m2row"] = nc.dram_tensor("m2row", [1, TOT], F16, kind="ExternalInput")
    din["aliveD"] = nc.dram_tensor("aliveD", [48, NVP], F16, kind="ExternalInput")
    din["bias48"] = nc.dram_tensor("bias48", [48, 1], F32, kind="ExternalInput")
    din["selSD"] = nc.dram_tensor("selSD", [51, 48], F32, kind="ExternalInput")
    din["nscale"] = nc.dram_tensor("nscale", [48, 1], F32, kind="ExternalInput")
    for name, shape in [("l1w", [96, 16]), ("l1b", [1, 16]),
                        ("l2w", [16, 4]), ("l2b", [1, 4]),
                        ("l3w", [4, 1]), ("l3b", [1, 1])]:
        din[name] = nc.dram_tensor(name, shape, F32, kind="ExternalInput")
    dout = nc.dram_tensor("out", [1], F32, kind="ExternalOutput")

    with tile.TileContext(nc) as tc:
        with (
            tc.tile_pool(name="stat", bufs=1) as stat,
            tc.tile_pool(name="wrk", bufs=2) as wrk,
            tc.tile_pool(name="tiny", bufs=2) as tiny,
            tc.tile_pool(name="dram", bufs=1, space="DRAM") as dram,
        ):
            idxW = stat.tile([128, TOT // 16], I16)
            aliveD = stat.tile([48, NVP], F16)
            bias48 = stat.tile([48, 1], F32)
            selS = stat.tile([51, 48], F32)
            nscale = stat.tile([48, 1], F32)
            l1w = stat.tile([96, 16], F32)
            l1b = stat.tile([1, 16], F32)
            l2w = stat.tile([16, 4], F32)
            l2b = stat.tile([1, 4], F32)
            l3w = stat.tile([4, 1], F32)
            l3b = stat.tile([1, 1], F32)
            for tl, name in [(idxW, "idxW"), (aliveD, "aliveD"),
                             (bias48, "bias48"), (selS, "selSD"), (nscale, "nscale"), (l1w, "l1w"), (l1b, "l1b"),
                             (l2w, "l2w"), (l2b, "l2b"), (l3w, "l3w"),
                             (l3b, "l3b")]:
                nc.sync.dma_start(out=tl[:], in_=din[name].ap())

            identF = stat.tile([128, 128], F16)
            make_identity(nc, identF[:])
            ident = stat.tile([128, 128], F32)
            make_identity(nc, ident[:])

            partials = stat.tile([128, 2], F32)

            for _rep in range(repeats):
                nc.vector.memset(partials[:], 0.0)
                with tc.tile_pool(name=f"ps{_rep}", bufs=2, space="PSUM") as psX:
                    off = 0
                    for b in range(NB):
                        widths, pad = blocks[b]
                        Wb = sum(widths) + pad
                        gA = wrk.tile([128, WMAX], F16, tag="gA", name="gA")
                        nc.gpsimd.dma_gather(
                            out_ap=gA[:, :Wb].rearrange("p (a w) -> p a w", a=1),
                            in_ap=din["tabD"].ap(),
                            idxs_ap=idxW[:, off // 16:(off + Wb) // 16],
                            num_idxs=Wb, num_idxs_reg=Wb, elem_size=128,
                            transpose=True, single_packet=False)
                        mrep = wrk.tile([49, WMAX], F16, tag="mrep", name="mrep")
                        nc.sync.dma_start(
                            out=mrep[32:49, :Wb],
                            in_=bass.AP(din["m2row"], off, [[0, 17], [1, Wb]]))
                        nc.vector.tensor_tensor(
                            out=gA[32:49, :Wb], in0=gA[32:49, :Wb],
                            in1=mrep[32:49, :Wb], op=OP.mult)

                        psU = psX.tile([128, BLK], F32, space="PSUM", tag="psU",
                                       name="psU")
                        r0 = 0
                        runs = [(r, w) for r, w in
                                zip(np.cumsum([0] + widths[:-1]), widths)]
                        if pad:
                            runs.append((sum(widths), pad))
                        nrun = len(runs)
                        for ri, (r, w) in enumerate(runs):
                            nc.tensor.matmul(
                                out=psU[:, :w], lhsT=identF[:],
                                rhs=gA[:, r:r + w],
                                start=(ri == 0), stop=(ri == nrun - 1),
                                skip_group_check=True)

                        # ---- tail ----
                        srec = tiny.tile([51, BLK], F32, tag="srec", name="srec")
                        nc.vector.tensor_scalar(
                            out=srec[32:51, :], in0=psU[32:51, :],
                            scalar1=1e-16, scalar2=None, op0=OP.add)
                        nc.vector.reciprocal(out=srec[32:51, :],
                                             in_=srec[32:51, :])
                        psB = psX.tile([48, BLK], F32, space="PSUM", tag="psB",
                                       name="psB")
                        nc.tensor.matmul(out=psB[:], lhsT=selS[32:51, :],
                                         rhs=srec[32:51, :], start=True,
                                         stop=True, skip_group_check=True)
                        hU = tiny.tile([48, BLK], F32, tag="hU", name="hU")
                        nc.vector.tensor_copy(out=hU[:], in_=psU[0:48, :])
                        nc.vector.tensor_tensor(out=hU[:], in0=hU[:],
                                                in1=psB[:], op=OP.mult)
                        nc.scalar.activation(out=hU[:], in_=hU[:], func=AF.Relu,
                                             bias=bias48[:])
                        nc.vector.tensor_tensor(
                            out=hU[:], in0=hU[:],
                            in1=aliveD[:, b * BLK:(b + 1) * BLK], op=OP.mult)
                        tred = tiny.tile([48, 2], F32, tag="tred", name="tred")
                        nc.vector.tensor_reduce(tred[:, 0:1], hU[:], AX.X, OP.add)
                        nc.vector.tensor_reduce(tred[:, 1:2], hU[:], AX.X, OP.max)
                        nc.vector.tensor_tensor(out=partials[0:48, 0:1],
                                                in0=partials[0:48, 0:1],
                                                in1=tred[:, 0:1], op=OP.add)
                        nc.vector.tensor_tensor(out=partials[0:48, 1:2],
                                                in0=partials[0:48, 1:2],
                                                in1=tred[:, 1:2], op=OP.max)
                        off += Wb

                # ---------------- combine + MLP ----------------
                with tc.tile_pool(name=f"psM{_rep}", bufs=1, space="PSUM") as psM:
                    def ps_small(shape, tg="tps"):
                        return psM.tile(shape, F32, space="PSUM", tag=tg,
                                        name="pstiny")

                    cc_in = dram.tile([128, 2], F32)
                    cc_out = dram.tile([NCORES * 128, 2], F32)
                    nc.sync.dma_start(out=cc_in[:], in_=partials[:])
                    nc.gpsimd.collective_compute(
                        "AllGather", OP.bypass,
                        replica_groups=[list(range(NCORES))],
                        ins=[cc_in[:].opt()], outs=[cc_out[:].opt()])
                    allp = tiny.tile([128, NCORES * 2], F32, tag="allp",
                                     name="allp")
                    nc.sync.dma_start(
                        out=allp[:],
                        in_=bass.AP(cc_out[:].tensor, cc_out[:].offset,
                                    [[2, 128], [256, NCORES], [1, 2]]))
                    comb = tiny.tile([128, 2], F32, tag="comb", name="comb")
                    ab = allp[:]
                    nc.vector.tensor_reduce(
                        comb[:, 0:1],
                        bass.AP(ab.tensor, ab.offset,
                                [list(ab.ap[0]), [1, 1], [2, NCORES]]),
                        AX.X, OP.add)
                    nc.vector.tensor_reduce(
                        comb[:, 1:2],
                        bass.AP(ab.tensor, ab.offset + 1,
                                [list(ab.ap[0]), [1, 1], [2, NCORES]]),
                        AX.X, OP.max)
                    nc.vector.tensor_tensor(out=comb[0:48, 0:1],
                                            in0=comb[0:48, 0:1],
                                            in1=nscale[:], op=OP.mult)
                    gcol = tiny.tile([96, 1], F32, tag="gcol", name="gcol")
                    for l in range(3):
                        nc.sync.dma_start(out=gcol[32 * l:32 * l + 16, :],
                                          in_=comb[16 * l:16 * l + 16, 0:1])
                        nc.sync.dma_start(out=gcol[32 * l + 16:32 * l + 32, :],
                                          in_=comb[16 * l:16 * l + 16, 1:2])
                    z1_ps = ps_small([1, 16])
                    nc.tensor.matmul(out=z1_ps[:], lhsT=gcol[:], rhs=l1w[:],
                                     start=True, stop=True)
                    h1 = tiny.tile([1, 16], F32, tag="h1", name="h1")
                    nc.vector.tensor_tensor(out=h1[:], in0=z1_ps[:], in1=l1b[:],
                                            op=OP.add)
                    nc.scalar.activation(out=h1[:], in_=h1[:], func=AF.Relu)
                    h1c_ps = ps_small([16, 1], tg="h1c")
                    nc.tensor.transpose(out=h1c_ps[:], in_=h1[:],
                                        identity=ident[:1, :1])
                    h1c = tiny.tile([16, 1], F32, tag="h1c", name="h1c")
                    nc.vector.tensor_copy(out=h1c[:], in_=h1c_ps[:])
                    z2_ps = ps_small([1, 4], tg="z2")
                    nc.tensor.matmul(out=z2_ps[:], lhsT=h1c[:], rhs=l2w[:],
                                     start=True, stop=True)
                    h2 = tiny.tile([1, 4], F32, tag="h2", name="h2")
                    nc.vector.tensor_tensor(out=h2[:], in0=z2_ps[:], in1=l2b[:],
                                            op=OP.add)
                    nc.scalar.activation(out=h2[:], in_=h2[:], func=AF.Relu)
                    h2c_ps = ps_small([4, 1], tg="h2c")
                    nc.tensor.transpose(out=h2c_ps[:], in_=h2[:],
                                        identity=ident[:1, :1])
                    h2c = tiny.tile([4, 1], F32, tag="h2c", name="h2c")
                    nc.vector.tensor_copy(out=h2c[:], in_=h2c_ps[:])
                    z3_ps = ps_small([1, 1], tg="z3")
                    nc.tensor.matmul(out=z3_ps[:], lhsT=h2c[:], rhs=l3w[:],
                                     start=True, stop=True)
                    z3 = tiny.tile([1, 1], F32, tag="z3", name="z3")
                    nc.vector.tensor_tensor(out=z3[:], in0=z3_ps[:], in1=l3b[:],
                                            op=OP.add)
                    sig = tiny.tile([1, 1], F32, tag="sig", name="sig")
                    nc.scalar.activation(out=sig[:], in_=z3[:], func=AF.Sigmoid)
                    nc.sync.dma_start(out=dout.ap(), in_=sig[:])

    nc.finalize()
    return nc


# ------------------------------------------------------------------ driver

_CACHE = {}


def _cache_key(meta):
    return (meta["TOT"], meta["NVP"],
            tuple((tuple(w), p) for w, p in meta["blocks"]))


def kernel(**inputs):
    in_maps_nc, meta = host_prep(**inputs)
    shared = pack_model_inputs(inputs)
    in_maps = [{**m, **shared} for m in in_maps_nc]

    key = _cache_key(meta)
    if key not in _CACHE:
        _CACHE[key] = build_program(meta)
    nc = _CACHE[key]

    res = run_bass_kernel_spmd(nc, in_maps, core_ids=list(range(NCORES)))
    return np.asarray(res.results[0]["out"], np.float32)


# revision 13
# speedup vs baseline: 4.6552x; 3.1882x over previous
"""Trainium2 Bass kernel for nn_DetectModel (RGAT x3 + TopK pool + MLP).

v2: SWDGE dma_gather (transpose mode) replaces gpsimd ap_gather.

Since x = emb[node_ids] has only 10 distinct rows and TopK scales are
per-attribute scalars, every per-edge quantity depends only on
c3 = (a_dst*114 + t)*10 + a_src (11400 combos).  A DRAM table holds, per
c3 and per alive1-state, a 128-plane row:
  planes 0:16 U0 = Mt0[s]*P0    16:32 U1*alive1    32:48 U2
  plane  48 P2   49 P0   50 P1   rest zero
(alive1-dead edges index a second bank of rows with U1/P1/U2/P2 zeroed;
alive2 masking is a DVE multiply on planes 32:49 with a host mask row.)

One dma_gather(transpose=True, elem=256B) per 512-vertex block fetches
each edge's row as a column: out[plane, edge].  Segment softmax sums
become PSUM accumulation over degree-prefix runs (vertices in each block
sorted by degree, so run k = first n_k columns — zero padding).  All
three layers' numerators and denominators accumulate in a single
[128, 512] PSUM tile per block; tails do recip/broadcast/relu/mask and
mean/max partials; one AllGather combines cores; tiny MLP finishes.

TopK keep-sets are replicated on host in numpy (scores take 10 distinct
values; selection is exact argsort replication of jax.lax.top_k).
"""
import numpy as np

import concourse.bass as bass
import concourse.bacc as bacc
import concourse.mybir as mybir
import concourse.tile as tile
from concourse.bass_utils import run_bass_kernel_spmd
from concourse.masks import make_identity

F32 = mybir.dt.float32
F16 = mybir.dt.float16
I16 = mybir.dt.int16
AF = mybir.ActivationFunctionType
OP = mybir.AluOpType
AX = mybir.AxisListType

N0, N1, N2 = 50000, 40000, 32000
E = 600000
D = 16
R = 114
NA = 10
NEG = 0.2
NCORES = 8

NC3 = R * NA * NA          # 11400
BANK = 11408               # state-0 rows start here
ZR = 2 * BANK              # all-zero row
NROWSP = ZR + 32           # padded table rows
BLK = 512                  # vertices per block / psum tile width


# ---------------------------------------------------------------- host prep

def host_prep(node_ids, edge_index, edge_type, emb, W0, q0, k0, W1, q1, k1,
              W2, q2, k2, pw0, pw1, **_unused):
    a = np.asarray(node_ids).astype(np.int64)
    src = np.asarray(edge_index)[0].astype(np.int64)
    dst = np.asarray(edge_index)[1].astype(np.int64)
    t = np.asarray(edge_type).astype(np.int64)
    emb = np.asarray(emb, np.float32)
    pw0 = np.asarray(pw0, np.float32).reshape(-1)
    pw1 = np.asarray(pw1, np.float32).reshape(-1)

    # ---- replicate reference TopK pooling on host (10 distinct scores) ----
    s0a = np.tanh((emb @ pw0) / np.linalg.norm(pw0))          # [10]
    score0 = s0a[a]
    perm1 = np.argsort(-score0, kind="stable")[:N1]
    keep1 = np.zeros(N0, bool)
    keep1[perm1] = True
    s1a = np.tanh(s0a * (emb @ pw1) / np.linalg.norm(pw1))    # [10]
    score1 = s1a[a[perm1]]
    perm2 = np.argsort(-score1, kind="stable")[:N2]
    keep2 = np.zeros(N0, bool)
    keep2[perm1[perm2]] = True

    m1 = keep1[src] & keep1[dst]
    m2 = m1 & keep2[src] & keep2[dst]

    # ---- per-edge fused index ----
    c3_e = (a[dst] * R + t) * NA + a[src]
    idx_e = np.where(m1, c3_e, c3_e + BANK).astype(np.int32)

    # ---- table [NROWSP, 128] f16 ----
    cc = np.stack([np.ones(NA, np.float32), s0a, s0a * s1a])  # [3, 10]
    tab = np.zeros((NROWSP, 128), np.float32)
    Ws = [np.asarray(W0, np.float32), np.asarray(W1, np.float32),
          np.asarray(W2, np.float32)]
    qs = [np.asarray(q0, np.float32).reshape(-1),
          np.asarray(q1, np.float32).reshape(-1),
          np.asarray(q2, np.float32).reshape(-1)]
    ks = [np.asarray(k0, np.float32).reshape(-1),
          np.asarray(k1, np.float32).reshape(-1),
          np.asarray(k2, np.float32).reshape(-1)]
    pslot = (49, 50, 48)   # P0, P1, P2 plane slots
    for l in range(3):
        xl = emb * cc[l][:, None]                              # [10, 16]
        XW = np.einsum("ad,tdk->tak", xl, Ws[l])               # [114, 10, 16]
        Mt = XW.reshape(R * NA, D)                             # s = t*10+a
        Tq = XW @ qs[l]                                        # [114, 10] (a_dst)
        Tk = (XW @ ks[l]).reshape(R * NA)                      # [1140] (s)
        z3 = (Tq.T[:, :, None] + Tk.reshape(1, R, NA)).reshape(NA, R * NA)
        P = np.exp(np.where(z3 > 0, z3, NEG * z3))             # [10, 1140]
        U = Mt[None, :, :] * P[:, :, None]                     # [10, 1140, 16]
        tab[:NC3, 16 * l:16 * l + 16] = U.reshape(NC3, D)
        tab[:NC3, pslot[l]] = P.reshape(NC3)
    # state-0 bank: only layer-0 planes survive
    tab[BANK:BANK + NC3, 0:16] = tab[:NC3, 0:16]
    tab[BANK:BANK + NC3, 49] = tab[:NC3, 49]
    tab16 = tab.astype(np.float16)

    # ---- vertex -> core assignment (degree snake) ----
    deg = np.bincount(dst, minlength=N0)
    e_order = np.argsort(dst, kind="stable")
    vstart = np.zeros(N0 + 1, np.int64)
    np.cumsum(deg, out=vstart[1:])

    vorder = np.argsort(-deg, kind="stable")
    nrows = (N0 + NCORES - 1) // NCORES
    vpad = np.full(nrows * NCORES, -1, np.int64)
    vpad[:N0] = vorder
    grid = vpad.reshape(nrows, NCORES)
    grid[1::2] = grid[1::2, ::-1]
    vlists = [grid[:, c][grid[:, c] >= 0] for c in range(NCORES)]
    nv = max(len(v) for v in vlists)
    NB = (nv + BLK - 1) // BLK
    NVP = NB * BLK

    # unified run widths per block: n_k[b][k] = max over cores
    degs = [deg[v] for v in vlists]
    blocks = []           # list of (widths list incl. first=BLK, pad)
    tot = 0
    for b in range(NB):
        maxd = 0
        for c in range(NCORES):
            db = degs[c][b * BLK:(b + 1) * BLK]
            if len(db):
                maxd = max(maxd, int(db.max()))
        widths = []
        for k in range(maxd):
            w = 0
            for c in range(NCORES):
                db = degs[c][b * BLK:(b + 1) * BLK]
                w = max(w, int((db > k).sum()))
            if w == 0:
                break
            widths.append(w)
        if not widths:
            widths = [1]
        widths[0] = BLK
        wsum = sum(widths)
        pad = (-wsum) % 128
        blocks.append((widths, pad))
        tot += wsum + pad
    TOT = tot

    # ---- per-core edge streams ----
    in_maps = []
    for c in range(NCORES):
        v = vlists[c]
        dgc = degs[c]
        idxs = np.full(TOT, ZR, np.int32)
        m2s = np.zeros(TOT, np.float16)
        pos = 0
        for b in range(NB):
            vb = v[b * BLK:(b + 1) * BLK]
            db = dgc[b * BLK:(b + 1) * BLK]
            widths, pad = blocks[b]
            for k, w in enumerate(widths):
                nreal = int((db > k).sum()) if len(db) else 0
                if nreal:
                    eids = e_order[vstart[vb[:nreal]] + k]
                    idxs[pos:pos + nreal] = idx_e[eids]
                    m2s[pos:pos + nreal] = m2[eids]
                pos += w
            pos += pad
        assert pos == TOT
        # wrap indices: flat j -> partition j%16, col j//16; replicate x8
        iw = np.zeros((16, TOT // 16), np.int16)
        iw[:, :] = idxs.reshape(TOT // 16, 16).T
        idxW = np.tile(iw, (8, 1))

        aliveD = np.zeros((48, NVP), np.float16)
        nreal_v = len(v)
        aliveD[0:16, :nreal_v] = 1.0
        aliveD[16:32, :nreal_v] = keep1[v][None, :]
        aliveD[32:48, :nreal_v] = keep2[v][None, :]

        in_maps.append({
            "idxW": np.ascontiguousarray(idxW),
            "m2row": np.ascontiguousarray(m2s.reshape(1, TOT)),
            "aliveD": np.ascontiguousarray(aliveD),
            "tabD": tab16,
        })

    meta = {"blocks": blocks, "TOT": TOT, "NVP": NVP, "NB": NB}
    return in_maps, meta


def pack_model_inputs(inp, meta=None):
    f = lambda x: np.ascontiguousarray(np.asarray(x, np.float32))
    b48 = np.zeros((48, 1), np.float32)
    for l in range(3):
        b48[16 * l:16 * l + 16, 0] = np.asarray(inp[f"b{l}"], np.float32)
    selS = np.zeros((51, 48), np.float32)
    selS[49, 0:16] = 1.0
    selS[50, 16:32] = 1.0
    selS[48, 32:48] = 1.0
    l1w = np.asarray(inp["l1w"], np.float32)
    A1 = np.zeros((128, 16), np.float32)
    A2 = np.zeros((128, 16), np.float32)
    for l, nl in enumerate((N0, N1, N2)):
        A1[16 * l:16 * l + 16] = l1w[32 * l:32 * l + 16] / nl
        A2[16 * l:16 * l + 16] = l1w[32 * l + 16:32 * l + 32]
    return {
        "bias48": b48, "selSD": selS, "A1": A1, "A2": A2,
        "l1w": f(inp["l1w"]), "l1b": f(inp["l1b"]).reshape(1, 16),
        "l2w": f(inp["l2w"]), "l2b": f(inp["l2b"]).reshape(1, 4),
        "l3w": f(inp["l3w"]), "l3b": f(inp["l3b"]).reshape(1, 1),
    }


# ------------------------------------------------------------- bass builder

def build_program(meta, repeats=1, num_devices=NCORES, skip=()):
    blocks, TOT, NVP, NB = (meta["blocks"], meta["TOT"], meta["NVP"],
                            meta["NB"])
    WMAX = max(sum(w for w in ws) + pad for ws, pad in blocks)

    nc = bacc.Bacc("TRN2", target_bir_lowering=False, debug=False,
                   num_devices=num_devices, num_swdge_queues=4)

    din = {}
    din["tabD"] = nc.dram_tensor("tabD", [NROWSP, 128], F16, kind="ExternalInput")
    din["idxW"] = nc.dram_tensor("idxW", [128, TOT // 16], I16, kind="ExternalInput")
    din["m2row"] = nc.dram_tensor("m2row", [1, TOT], F16, kind="ExternalInput")
    din["aliveD"] = nc.dram_tensor("aliveD", [48, NVP], F16, kind="ExternalInput")
    din["bias48"] = nc.dram_tensor("bias48", [48, 1], F32, kind="ExternalInput")
    din["selSD"] = nc.dram_tensor("selSD", [51, 48], F32, kind="ExternalInput")
    din["A1"] = nc.dram_tensor("A1", [128, 16], F32, kind="ExternalInput")
    din["A2"] = nc.dram_tensor("A2", [128, 16], F32, kind="ExternalInput")
    for name, shape in [("l1w", [96, 16]), ("l1b", [1, 16]),
                        ("l2w", [16, 4]), ("l2b", [1, 4]),
                        ("l3w", [4, 1]), ("l3b", [1, 1])]:
        din[name] = nc.dram_tensor(name, shape, F32, kind="ExternalInput")
    dout = nc.dram_tensor("out", [1], F32, kind="ExternalOutput")

    with tile.TileContext(nc) as tc:
        with (
            tc.tile_pool(name="stat", bufs=1) as stat,
            tc.tile_pool(name="wrk", bufs=4) as wrk,
            tc.tile_pool(name="tiny", bufs=2) as tiny,
            tc.tile_pool(name="dram", bufs=1, space="DRAM") as dram,
        ):
            idxW = stat.tile([128, TOT // 16], I16)
            aliveD = stat.tile([48, NVP], F16)
            bias48 = stat.tile([48, 1], F32)
            selS = stat.tile([51, 48], F32)
            A1 = stat.tile([128, 16], F32)
            A2 = stat.tile([128, 16], F32)
            l1b = stat.tile([1, 16], F32)
            l2w = stat.tile([16, 4], F32)
            l2b = stat.tile([1, 4], F32)
            l3w = stat.tile([4, 1], F32)
            l3b = stat.tile([1, 1], F32)
            for tl, name in [(idxW, "idxW"), (aliveD, "aliveD"),
                             (bias48, "bias48"), (selS, "selSD"), (A1, "A1"), (A2, "A2"), (l1b, "l1b"),
                             (l2w, "l2w"), (l2b, "l2b"), (l3w, "l3w"),
                             (l3b, "l3b")]:
                nc.sync.dma_start(out=tl[:], in_=din[name].ap())

            identF = stat.tile([128, 128], F16)
            make_identity(nc, identF[:])
            ident = stat.tile([128, 128], F32)
            make_identity(nc, ident[:])

            partials2 = [stat.tile([128, 2], F32, tag=f"part{i}",
                                   name=f"part{i}") for i in range(2)]

            for _rep in range(repeats):
                partials = partials2[_rep % 2]
                nc.vector.memset(partials[:], 0.0)
                with tc.tile_pool(name=f"ps{_rep}", bufs=2, space="PSUM") as psX:
                    off = 0
                    for b in range(NB):
                        widths, pad = blocks[b]
                        Wb = sum(widths) + pad
                        gA = wrk.tile([128, WMAX], F16, tag="gA", name="gA")
                        if "gather" not in skip:
                            n128 = Wb // 128
                            cuts = [0] + [128 * ((n128 * i) // 4)
                                          for i in range(1, 4)] + [Wb]
                            for i in range(4):
                                g0, g1 = cuts[i], cuts[i + 1]
                                if g1 == g0:
                                    continue
                                nc.gpsimd.dma_gather(
                                    out_ap=gA[:, g0:g1].rearrange(
                                        "p (a w) -> p a w", a=1),
                                    in_ap=din["tabD"].ap(),
                                    idxs_ap=idxW[:, (off + g0) // 16:
                                                 (off + g1) // 16],
                                    num_idxs=g1 - g0, num_idxs_reg=g1 - g0,
                                    elem_size=128, transpose=True,
                                    single_packet=False,
                                    queue_num=(b + i) % 4)
                        if "mask" not in skip:
                            mrep = wrk.tile([49, WMAX], F16, tag="mrep", name="mrep")
                            nc.sync.dma_start(
                                out=mrep[32:49, :Wb],
                                in_=bass.AP(din["m2row"], off, [[0, 17], [1, Wb]]))
                            nc.vector.tensor_tensor(
                                out=gA[32:49, :Wb], in0=gA[32:49, :Wb],
                                in1=mrep[32:49, :Wb], op=OP.mult)

                        psU = psX.tile([128, BLK], F32, space="PSUM", tag="psU",
                                       name="psU")
                        r0 = 0
                        runs = [(r, w) for r, w in
                                zip(np.cumsum([0] + widths[:-1]), widths)]
                        if pad:
                            runs.append((sum(widths), pad))
                        nrun = len(runs)
                        for ri, (r, w) in enumerate(runs):
                            if "pe" in skip and 0 < ri < nrun - 1:
                                continue
                            nc.tensor.matmul(
                                out=psU[:, :w], lhsT=identF[:],
                                rhs=gA[:, r:r + w],
                                start=(ri == 0), stop=(ri == nrun - 1),
                                skip_group_check=True)

                        # ---- tail ----
                        if "tail" in skip:
                            off += Wb
                            continue
                        srec = tiny.tile([51, BLK], F32, tag="srec", name="srec")
                        nc.vector.tensor_scalar(
                            out=srec[32:51, :], in0=psU[32:51, :],
                            scalar1=1e-16, scalar2=None, op0=OP.add)
                        nc.vector.reciprocal(out=srec[32:51, :],
                                             in_=srec[32:51, :])
                        psB = psX.tile([48, BLK], F32, space="PSUM", tag="psB",
                                       name="psB")
                        nc.tensor.matmul(out=psB[:], lhsT=selS[32:51, :],
                                         rhs=srec[32:51, :], start=True,
                                         stop=True, skip_group_check=True)
                        hU = tiny.tile([48, BLK], F32, tag="hU", name="hU")
                        nc.vector.tensor_copy(out=hU[:], in_=psU[0:48, :])
                        nc.vector.tensor_tensor(out=hU[:], in0=hU[:],
                                                in1=psB[:], op=OP.mult)
                        nc.scalar.activation(out=hU[:], in_=hU[:], func=AF.Relu,
                                             bias=bias48[:])
                        nc.vector.tensor_tensor(
                            out=hU[:], in0=hU[:],
                            in1=aliveD[:, b * BLK:(b + 1) * BLK], op=OP.mult)
                        tred = tiny.tile([48, 2], F32, tag="tred", name="tred")
                        nc.vector.tensor_reduce(tred[:, 0:1], hU[:], AX.X, OP.add)
                        nc.vector.tensor_reduce(tred[:, 1:2], hU[:], AX.X, OP.max)
                        nc.vector.tensor_tensor(out=partials[0:48, 0:1],
                                                in0=partials[0:48, 0:1],
                                                in1=tred[:, 0:1], op=OP.add)
                        nc.vector.tensor_tensor(out=partials[0:48, 1:2],
                                                in0=partials[0:48, 1:2],
                                                in1=tred[:, 1:2], op=OP.max)
                        off += Wb

                # ---------------- combine + MLP ----------------
                if "cc" in skip:
                    nc.sync.dma_start(out=dout.ap(), in_=partials[:1, :1])
                    continue
                with tc.tile_pool(name=f"psM{_rep}", bufs=1, space="PSUM") as psM:
                    def ps_small(shape, tg="tps"):
                        return psM.tile(shape, F32, space="PSUM", tag=tg,
                                        name="pstiny")

                    cc_in = dram.tile([128, 2], F32)
                    cc_out = dram.tile([NCORES * 128, 2], F32)
                    nc.sync.dma_start(out=cc_in[:], in_=partials[:])
                    allp = tiny.tile([128, NCORES * 2], F32, tag="allp",
                                     name="allp")
                    if "coll" in skip:
                        nc.sync.dma_start(
                            out=allp[:],
                            in_=bass.AP(cc_in[:].tensor, cc_in[:].offset,
                                        [[2, 128], [0, NCORES], [1, 2]]))
                    else:
                        nc.gpsimd.collective_compute(
                            "AllGather", OP.bypass,
                            replica_groups=[list(range(NCORES))],
                            ins=[cc_in[:].opt()], outs=[cc_out[:].opt()])
                        nc.sync.dma_start(
                            out=allp[:],
                            in_=bass.AP(cc_out[:].tensor, cc_out[:].offset,
                                        [[2, 128], [256, NCORES], [1, 2]]))
                    comb = tiny.tile([128, 2], F32, tag="comb", name="comb")
                    ab = allp[:]
                    nc.vector.tensor_reduce(
                        comb[:, 0:1],
                        bass.AP(ab.tensor, ab.offset,
                                [list(ab.ap[0]), [1, 1], [2, NCORES]]),
                        AX.X, OP.add)
                    nc.vector.tensor_reduce(
                        comb[:, 1:2],
                        bass.AP(ab.tensor, ab.offset + 1,
                                [list(ab.ap[0]), [1, 1], [2, NCORES]]),
                        AX.X, OP.max)
                    z1_ps = ps_small([1, 16])
                    nc.tensor.matmul(out=z1_ps[:], lhsT=comb[:, 0:1],
                                     rhs=A1[:], start=True, stop=False,
                                     skip_group_check=True)
                    nc.tensor.matmul(out=z1_ps[:], lhsT=comb[:, 1:2],
                                     rhs=A2[:], start=False, stop=True,
                                     skip_group_check=True)
                    h1 = tiny.tile([1, 16], F32, tag="h1", name="h1")
                    nc.vector.tensor_tensor(out=h1[:], in0=z1_ps[:], in1=l1b[:],
                                            op=OP.add)
                    nc.scalar.activation(out=h1[:], in_=h1[:], func=AF.Relu)
                    h1c_ps = ps_small([16, 1], tg="h1c")
                    nc.tensor.transpose(out=h1c_ps[:], in_=h1[:],
                                        identity=ident[:1, :1])
                    h1c = tiny.tile([16, 1], F32, tag="h1c", name="h1c")
                    nc.vector.tensor_copy(out=h1c[:], in_=h1c_ps[:])
                    z2_ps = ps_small([1, 4], tg="z2")
                    nc.tensor.matmul(out=z2_ps[:], lhsT=h1c[:], rhs=l2w[:],
                                     start=True, stop=True)
                    h2 = tiny.tile([1, 4], F32, tag="h2", name="h2")
                    nc.vector.tensor_tensor(out=h2[:], in0=z2_ps[:], in1=l2b[:],
                                            op=OP.add)
                    nc.scalar.activation(out=h2[:], in_=h2[:], func=AF.Relu)
                    h2c_ps = ps_small([4, 1], tg="h2c")
                    nc.tensor.transpose(out=h2c_ps[:], in_=h2[:],
                                        identity=ident[:1, :1])
                    h2c = tiny.tile([4, 1], F32, tag="h2c", name="h2c")
                    nc.vector.tensor_copy(out=h2c[:], in_=h2c_ps[:])
                    z3_ps = ps_small([1, 1], tg="z3")
                    nc.tensor.matmul(out=z3_ps[:], lhsT=h2c[:], rhs=l3w[:],
                                     start=True, stop=True)
                    z3 = tiny.tile([1, 1], F32, tag="z3", name="z3")
                    nc.vector.tensor_tensor(out=z3[:], in0=z3_ps[:], in1=l3b[:],
                                            op=OP.add)
                    sig = tiny.tile([1, 1], F32, tag="sig", name="sig")
                    nc.scalar.activation(out=sig[:], in_=z3[:], func=AF.Sigmoid)
                    nc.sync.dma_start(out=dout.ap(), in_=sig[:])

    nc.finalize()
    return nc


# ------------------------------------------------------------------ driver

_CACHE = {}


def _cache_key(meta):
    return (meta["TOT"], meta["NVP"],
            tuple((tuple(w), p) for w, p in meta["blocks"]))


def kernel(**inputs):
    in_maps_nc, meta = host_prep(**inputs)
    shared = pack_model_inputs(inputs)
    in_maps = [{**m, **shared} for m in in_maps_nc]

    key = _cache_key(meta)
    if key not in _CACHE:
        _CACHE[key] = build_program(meta)
    nc = _CACHE[key]

    res = run_bass_kernel_spmd(nc, in_maps, core_ids=list(range(NCORES)))
    return np.asarray(res.results[0]["out"], np.float32)
